# revision 1
# baseline (speedup 1.0000x reference)
"""RNN-T joint network kernel for 8 Trainium2 NeuronCores.

out[b,t,u,:] = W2 @ tanh(W1e @ enc[b,t] + W1d @ dec[b,u] + b1) + b2

Shapes: B=4, T=200, U=100, D=512, H=1024, O=512 (all fp32).
Sharding: T split 8 ways (25 t's per core); dec + weights replicated.

Per-core device program:
  Phase 1: enc_hT[h, b*25+t] = W1e @ encT (+b1), dec_hT[h, b*100+u] = W1d @ decT
           (h on partitions in 8 chunks of 128; small matmuls).
  Phase 2: for each chunk (b, 5 t's) = 500 rows:
           s[kchunk, t, u] = dec_hT[k][:, b-block] (+) enc_hT broadcast  (DVE,
           stride-0 broadcast APs), tanh over the whole [128, 4000] tile (ACT),
           then 4x8 accumulating matmuls against W2T chunks -> psum [128, 512],
           add b2 + copy to SBUF (DVE), DMA out.
"""

from contextlib import ExitStack

import numpy as np

import concourse.bacc as bacc
import concourse.bass as bass
import concourse.mybir as mybir
import concourse.tile as tile
from concourse.bass_utils import run_bass_kernel_spmd

F32 = mybir.dt.float32
F32R = mybir.dt.float32r

B, T, U, D, H, O = 4, 200, 100, 512, 1024, 512
NCORES = 8
TLOC = T // NCORES            # 25 t's per core
PAIRS = B * TLOC              # 100 (b,t) pairs per core
TCH = 5                       # t's per inner chunk
CHROWS = TCH * U              # 500 rows per chunk
NCH = TLOC // TCH             # 5 chunks per b
ROWS = PAIRS * U              # 10000 output rows per core
DK = D // 128                 # 4 contraction chunks for phase 1
HK = H // 128                 # 8 h chunks

_CACHE = {}


def _build():
    nc = bacc.Bacc("TRN2", target_bir_lowering=False, debug=False,
                   num_devices=NCORES)
    # inputs arrive pre-interleaved in SBUF layout: [128, nchunk*width],
    # partition p holding chunk k's row (k*128+p) at cols [k*width, ...)
    encT = nc.dram_tensor("encT", [128, DK * PAIRS], F32, kind="ExternalInput")
    decT = nc.dram_tensor("decT", [128, DK * B * U], F32, kind="ExternalInput")
    w1eT = nc.dram_tensor("w1eT", [128, DK * H], F32, kind="ExternalInput")
    w1dT = nc.dram_tensor("w1dT", [128, DK * H], F32, kind="ExternalInput")
    w2T = nc.dram_tensor("w2T", [128, HK * O], F32, kind="ExternalInput")
    b1r = nc.dram_tensor("b1r", [128, HK], F32, kind="ExternalInput")
    b2c = nc.dram_tensor("b2c", [128, O // 128], F32, kind="ExternalInput")
    out = nc.dram_tensor("out", [O, ROWS], F32, kind="ExternalOutput")

    BU = B * U
    with tile.TileContext(nc) as tc, ExitStack() as ctx:
        consts = ctx.enter_context(tc.tile_pool(name="consts", bufs=1))
        spool = ctx.enter_context(tc.tile_pool(name="spool", bufs=4))
        opool = ctx.enter_context(tc.tile_pool(name="opool", bufs=8))
        psB = ctx.enter_context(tc.tile_pool(name="psB", bufs=8, space="PSUM"))

        # ---- load constants / inputs ----
        w1e_s = consts.tile([128, DK * H], F32)      # dk-chunk k at cols [k*H, (k+1)*H)
        w1d_s = consts.tile([128, DK * H], F32)
        w2_s = consts.tile([128, HK * O], F32)       # hk-chunk k at cols [k*O, (k+1)*O)
        encT_s = consts.tile([128, DK * PAIRS], F32)
        decT_s = consts.tile([128, DK * BU], F32)
        b1_s = consts.tile([128, HK], F32)
        b2c_s = consts.tile([128, O // 128], F32)
        # split loads across the two HWDGE rings (sync + scalar) so the
        # enc-side and dec-side transfers run in parallel; all plain 2D
        # contiguous DMAs (inputs are pre-interleaved on the host)
        nc.sync.dma_start(encT_s[:], encT[:])
        nc.scalar.dma_start(decT_s[:], decT[:])
        nc.sync.dma_start(w1e_s[:], w1eT[:])
        nc.scalar.dma_start(w1d_s[:], w1dT[:])
        nc.sync.dma_start(w2_s[:], w2T[:])
        nc.scalar.dma_start(b1_s[:], b1r[:])
        nc.scalar.dma_start(b2c_s[:], b2c[:])

        # float32r copies (fp32r matmul inputs must come from rounding
        # producers; DMA does not qualify)
        w1e_r = consts.tile([128, DK * H], F32R)
        w1d_r = consts.tile([128, DK * H], F32R)
        w2_r = consts.tile([128, HK * O], F32R)
        encT_r = consts.tile([128, DK * PAIRS], F32R)
        decT_r = consts.tile([128, DK * BU], F32R)
        nc.vector.tensor_copy(encT_r[:], encT_s[:])
        nc.vector.tensor_copy(w1e_r[:], w1e_s[:])
        nc.vector.tensor_copy(decT_r[:], decT_s[:])
        nc.vector.tensor_copy(w1d_r[:], w1d_s[:])
        nc.vector.tensor_copy(w2_r[:], w2_s[:])

        # ---- phase 1: enc_hT (+b1) and dec_hT ----
        # per-k tiles so phase-2 builds can start as soon as *their* k chunk
        # is ready (a single big tile would serialize phase 2 behind all of
        # phase 1 via coarse dependency tracking)
        ench_t = [consts.tile([128, PAIRS], F32, name=f"ench{k}") for k in range(HK)]
        dech_t = [consts.tile([128, BU], F32, name=f"dech{k}") for k in range(HK)]
        # enc matmuls first: they only need encT+w1e, and cover the
        # decT/w1d DMA + cast latency with PE work
        for hk in range(HK):
            pe = psB.tile([128, 512], F32, tag="psB", name="pe")
            pe = pe[:, :PAIRS]
            for dk in range(DK):
                nc.tensor.matmul(
                    pe[:],
                    lhsT=w1e_r[:, dk * H + hk * 128: dk * H + (hk + 1) * 128],
                    rhs=encT_r[:, dk * PAIRS:(dk + 1) * PAIRS],
                    start=(dk == 0), stop=(dk == DK - 1),
                )
            nc.vector.tensor_scalar_add(ench_t[hk][:], pe[:], b1_s[:, hk:hk + 1])
        for hk in range(HK):
            pd = psB.tile([128, 512], F32, tag="psB", name="pd")
            pd = pd[:, :BU]
            for dk in range(DK):
                nc.tensor.matmul(
                    pd[:],
                    lhsT=w1d_r[:, dk * H + hk * 128: dk * H + (hk + 1) * 128],
                    rhs=decT_r[:, dk * BU:(dk + 1) * BU],
                    start=(dk == 0), stop=(dk == DK - 1),
                )
            nc.vector.tensor_copy(dech_t[hk][:], pd[:])

        # ---- phase 2: chunks of (b, up to 5 t's) ----
        # small leading chunks shorten the build+tanh fill before the first
        # big matmul group
        chunks = []
        for b in range(B):
            sizes = [1, 4] + [TCH] * 4 if b == 0 else [TCH] * NCH
            t0c = 0
            for tch in sizes:
                chunks.append((b, t0c, tch))
                t0c += tch
        for b, t0c, tch in chunks:
            rows_c = tch * U
            s_t = spool.tile([128, HK * CHROWS], F32R, tag="s")
            for k in range(HK):
                in0 = dech_t[k][:, b * U:(b + 1) * U]
                in0 = in0.rearrange("p (a u) -> p a u", a=1)
                c0 = b * TLOC + t0c
                in1 = ench_t[k][:, c0:c0 + tch].rearrange("p (t a) -> p t a", a=1)
                bc0, bc1 = bass.broadcast_tensor_aps(in0, in1)
                outap = s_t[:, k * CHROWS: k * CHROWS + rows_c].rearrange(
                    "p (t u) -> p t u", t=tch)
                nc.vector.tensor_tensor(outap, bc0, bc1, mybir.AluOpType.add)
            s_used = s_t[:].rearrange("p (k c) -> p k c", k=HK)[:, :, :rows_c]
            nc.scalar.activation(s_used, s_used,
                                 mybir.ActivationFunctionType.Tanh)
            row0 = b * (TLOC * U) + t0c * U
            # swapped matmul: W2 blocks stationary, s moving -> psum holds
            # out^T [o-chunk, rows]; b2 folds into the psum->sbuf copy as a
            # per-partition bias.
            for oc in range(O // 128):
                ps = psB.tile([128, 512], F32, tag="psB")
                for k in range(HK):
                    nc.tensor.matmul(
                        ps[:, :rows_c],
                        lhsT=w2_r[:, k * O + oc * 128: k * O + (oc + 1) * 128],
                        rhs=s_t[:, k * CHROWS: k * CHROWS + rows_c],
                        start=(k == 0), stop=(k == HK - 1),
                    )
                ot = opool.tile([128, CHROWS], F32, tag="ot")
                if oc < 2:
                    nc.scalar.activation(
                        ot[:, :rows_c], ps[:, :rows_c],
                        mybir.ActivationFunctionType.Identity,
                        bias=b2c_s[:, oc:oc + 1])
                else:
                    nc.vector.tensor_scalar_add(
                        ot[:, :rows_c], ps[:, :rows_c], b2c_s[:, oc:oc + 1])
                nc.sync.dma_start(
                    out[oc * 128:(oc + 1) * 128, row0:row0 + rows_c],
                    ot[:, :rows_c])
    nc.compile()
    return nc


def kernel(enc_state, dec_state, W1, b1, W2, b2, _trace=False):
    enc_state = np.ascontiguousarray(enc_state, dtype=np.float32)
    dec_state = np.ascontiguousarray(dec_state, dtype=np.float32)
    W1 = np.asarray(W1, dtype=np.float32)
    b1 = np.asarray(b1, dtype=np.float32)
    W2 = np.asarray(W2, dtype=np.float32)
    b2 = np.asarray(b2, dtype=np.float32)

    if "nc" not in _CACHE:
        _CACHE["nc"] = _build()
    nc = _CACHE["nc"]

    def chunk128(a):
        # [n*128, w] -> [128, n*w]: partition p holds row k*128+p of chunk k
        n = a.shape[0] // 128
        return np.ascontiguousarray(
            a.reshape(n, 128, a.shape[1]).transpose(1, 0, 2).reshape(128, -1))

    decT = chunk128(dec_state.reshape(B * U, D).T)                      # [128, 4*400]
    w1eT = chunk128(W1[:, :D].T)                                        # [128, 4*H]
    w1dT = chunk128(W1[:, D:].T)                                        # [128, 4*H]
    w2T = chunk128(W2.T)                                                # [128, 8*O]
    b1r = np.ascontiguousarray(b1.reshape(HK, 128).T)                   # [128, HK]
    b2cm = np.ascontiguousarray(b2.reshape(O // 128, 128).T)            # [128, 4]

    in_maps = []
    for c in range(NCORES):
        enc_c = enc_state[:, c * TLOC:(c + 1) * TLOC, :].reshape(PAIRS, D)
        encT_c = chunk128(enc_c.T)                                      # [128, 4*100]
        in_maps.append({
            "encT": encT_c, "decT": decT, "w1eT": w1eT, "w1dT": w1dT,
            "w2T": w2T, "b1r": b1r, "b2c": b2cm,
        })

    res = run_bass_kernel_spmd(nc, in_maps, list(range(NCORES)), trace=_trace)
    out = np.empty((B, T, U, O), dtype=np.float32)
    for c in range(NCORES):
        # device output is transposed: [O, ROWS]
        out[:, c * TLOC:(c + 1) * TLOC] = (
            res.results[c]["out"].T.reshape(B, TLOC, U, O))
    if _trace:
        kernel.last_results = res
    return out



# revision 2
# speedup vs baseline: 1.3435x; 1.3435x over previous
"""RNN-T joint network kernel for 8 Trainium2 NeuronCores.

out[b,t,u,:] = W2 @ tanh(W1e @ enc[b,t] + W1d @ dec[b,u] + b1) + b2

Shapes: B=4, T=200, U=100, D=512, H=1024, O=512 (fp32 in/out).
Sharding: T split 8 ways (25 t's per core); dec + weights replicated.

All device compute is bf16 (inputs cast on host; ~5e-4 rel err, well
under the 2e-2 gate). bf16 matmuls stream at the same 1 cycle/row as
fp32r but halve LDWEIGHTS via fast-weight-load, halve the input DMA
bytes, and remove the on-device fp32->fp32r cast pass entirely.

Per-core device program:
  Phase 1: W1 split into 4 per-dk weight tiles per side so the first
           matmul can issue as soon as the first 256KB of weights lands.
           enc matmuls run dk-outer into 8 packed psum banks (enc in
           cols 0:100, dec in cols 100:500 of the same bank); dec runs
           hk-outer so the psum->sbuf copies (+b1 on the enc half)
           pipeline behind the remaining dec matmul groups.
  Phase 2: per chunk (b, up to 5 t's): two fused broadcast-add builds
           (4 h-chunks each, [p, k, t, u] APs) on DVE -> bf16 s tile,
           one tanh over [128, 8*rows] on ACT, then 4x8 accumulating
           bf16 matmuls -> psum out^T chunks, bias-add copies split
           ACT/DVE, output DMAs split across both HWDGE rings.
"""

from contextlib import ExitStack

import numpy as np
import ml_dtypes

import concourse.bacc as bacc
import concourse.bass as bass
import concourse.mybir as mybir
import concourse.tile as tile
from concourse.bass_utils import run_bass_kernel_spmd

F32 = mybir.dt.float32
BF16 = mybir.dt.bfloat16
BF16NP = ml_dtypes.bfloat16

B, T, U, D, H, O = 4, 200, 100, 512, 1024, 512
NCORES = 8
TLOC = T // NCORES            # 25 t's per core
PAIRS = B * TLOC              # 100 (b,t) pairs per core
TCH = 5                       # t's per inner chunk
CHROWS = TCH * U              # 500 rows per chunk
NCH = TLOC // TCH             # 5 chunks per b
ROWS = PAIRS * U              # 10000 output rows per core
DK = D // 128                 # 4 contraction chunks for phase 1
HK = H // 128                 # 8 h chunks
BU = B * U                    # 400

_CACHE = {}


def _build():
    nc = bacc.Bacc("TRN2", target_bir_lowering=False, debug=False,
                   num_devices=NCORES)
    # inputs arrive pre-interleaved in SBUF layout: [128, nchunk*width],
    # partition p holding chunk k's row (k*128+p) at cols [k*width, ...)
    encT = nc.dram_tensor("encT", [128, DK * PAIRS], BF16, kind="ExternalInput")
    decT = nc.dram_tensor("decT", [128, DK * BU], BF16, kind="ExternalInput")
    w1eT = nc.dram_tensor("w1eT", [128, DK * H], BF16, kind="ExternalInput")
    w1dT = nc.dram_tensor("w1dT", [128, DK * H], BF16, kind="ExternalInput")
    w2T = nc.dram_tensor("w2T", [128, HK * O], BF16, kind="ExternalInput")
    b1r = nc.dram_tensor("b1r", [128, HK], F32, kind="ExternalInput")
    b2c = nc.dram_tensor("b2c", [128, O // 128], F32, kind="ExternalInput")
    out = nc.dram_tensor("out", [O, ROWS], F32, kind="ExternalOutput")

    with tile.TileContext(nc) as tc, ExitStack() as ctx:
        consts = ctx.enter_context(tc.tile_pool(name="consts", bufs=1))
        spool = ctx.enter_context(tc.tile_pool(name="spool", bufs=4))
        opool = ctx.enter_context(tc.tile_pool(name="opool", bufs=8))
        psB = ctx.enter_context(tc.tile_pool(name="psB", bufs=8, space="PSUM"))

        encT_s = consts.tile([128, DK * PAIRS], BF16)
        decT_s = consts.tile([128, DK * BU], BF16)
        w1e_t = [consts.tile([128, H], BF16, name=f"w1e{k}") for k in range(DK)]
        w1d_t = [consts.tile([128, H], BF16, name=f"w1d{k}") for k in range(DK)]
        w2_s = consts.tile([128, HK * O], BF16)
        b1_s = consts.tile([128, HK], F32)
        b2c_s = consts.tile([128, O // 128], F32)
        ench_f = consts.tile([128, HK * PAIRS], BF16)
        dech_A = consts.tile([128, 4 * BU], BF16)
        dech_B = consts.tile([128, 4 * BU], BF16)

        # sync ring: enc-side path (needed first) then W2;
        # scalar ring: small biases, dec-side path.
        nc.sync.dma_start(encT_s[:], encT[:])
        for k in range(DK):
            nc.sync.dma_start(w1e_t[k][:], w1eT[:, k * H:(k + 1) * H])
        nc.sync.dma_start(w2_s[:], w2T[:])
        nc.scalar.dma_start(b1_s[:], b1r[:])
        nc.scalar.dma_start(b2c_s[:], b2c[:])
        nc.scalar.dma_start(decT_s[:], decT[:])
        for k in range(DK):
            nc.scalar.dma_start(w1d_t[k][:], w1dT[:, k * H:(k + 1) * H])

        # ---- phase 1 ----
        # 8 psum banks, each packing enc (cols 0:100) + dec (cols 100:500)
        # for one h-chunk. enc runs dk-outer: the dk0 matmuls only need the
        # first w1e tile, so the PE starts ~1us in.
        ph = [psB.tile([128, 512], F32, tag="psB", name=f"ph{hk}")
              for hk in range(HK)]
        for dk in range(DK):
            for hk in range(HK):
                nc.tensor.matmul(
                    ph[hk][:, 0:PAIRS],
                    lhsT=w1e_t[dk][:, hk * 128:(hk + 1) * 128],
                    rhs=encT_s[:, dk * PAIRS:(dk + 1) * PAIRS],
                    start=(dk == 0), stop=(dk == DK - 1),
                )
        for hk in range(HK):
            for dk in range(DK):
                nc.tensor.matmul(
                    ph[hk][:, PAIRS:PAIRS + BU],
                    lhsT=w1d_t[dk][:, hk * 128:(hk + 1) * 128],
                    rhs=decT_s[:, dk * BU:(dk + 1) * BU],
                    start=(dk == 0), stop=(dk == DK - 1),
                )
            # copies chase the dec matmul groups hk by hk
            nc.vector.tensor_scalar_add(
                ench_f[:, hk * PAIRS:(hk + 1) * PAIRS],
                ph[hk][:, 0:PAIRS], b1_s[:, hk:hk + 1])
            dst = dech_A if hk < 4 else dech_B
            nc.vector.tensor_copy(
                dst[:, (hk % 4) * BU:(hk % 4 + 1) * BU],
                ph[hk][:, PAIRS:PAIRS + BU])

        # ---- phase 2 ----
        ench_v = ench_f[:].rearrange("p (k t a) -> p k t a", k=HK, a=1)
        dech_vA = dech_A[:].rearrange("p (k a u) -> p k a u", k=4, a=1)
        dech_vB = dech_B[:].rearrange("p (k a u) -> p k a u", k=4, a=1)
        chunks = []
        for b in range(B):
            if b == 0:
                sizes = [1, 4, 5, 5, 5, 5]
            elif b == B - 1:
                sizes = [5, 5, 5, 5, 4, 1]
            else:
                sizes = [TCH] * NCH
            t0c = 0
            for tch in sizes:
                chunks.append((b, t0c, tch))
                t0c += tch
        for b, t0c, tch in chunks:
            rows_c = tch * U
            s_t = spool.tile([128, HK * CHROWS], BF16, tag="s")
            sv = s_t[:].rearrange("p (k t u) -> p k t u", k=HK, t=TCH)
            for half, dech_v in ((0, dech_vA), (1, dech_vB)):
                in0 = dech_v[:, :, :, b * U:(b + 1) * U]            # [p,4,1,100]
                c0 = b * TLOC + t0c
                in1 = ench_v[:, half * 4:(half + 1) * 4, c0:c0 + tch, :]
                bc0, bc1 = bass.broadcast_tensor_aps(in0, in1)
                outap = sv[:, half * 4:(half + 1) * 4, 0:tch, :]
                nc.vector.tensor_tensor(outap, bc0, bc1, mybir.AluOpType.add)
            s_used = s_t[:].rearrange("p (k c) -> p k c", k=HK)[:, :, :rows_c]
            nc.scalar.activation(s_used, s_used,
                                 mybir.ActivationFunctionType.Tanh)
            row0 = b * (TLOC * U) + t0c * U
            # swapped matmul: W2 blocks stationary, s moving -> psum holds
            # out^T [o-chunk, rows]; b2 folds into the psum->sbuf copy.
            for oc in range(O // 128):
                ps = psB.tile([128, 512], F32, tag="psB")
                for k in range(HK):
                    nc.tensor.matmul(
                        ps[:, :rows_c],
                        lhsT=w2_s[:, k * O + oc * 128: k * O + (oc + 1) * 128],
                        rhs=s_t[:, k * CHROWS: k * CHROWS + rows_c],
                        start=(k == 0), stop=(k == HK - 1),
                    )
                ot = opool.tile([128, CHROWS], F32, tag="ot")
                if oc < 2:
                    nc.scalar.activation(
                        ot[:, :rows_c], ps[:, :rows_c],
                        mybir.ActivationFunctionType.Identity,
                        bias=b2c_s[:, oc:oc + 1])
                else:
                    nc.vector.tensor_scalar_add(
                        ot[:, :rows_c], ps[:, :rows_c], b2c_s[:, oc:oc + 1])
                ring = nc.sync if oc % 2 == 0 else nc.scalar
                ring.dma_start(
                    out[oc * 128:(oc + 1) * 128, row0:row0 + rows_c],
                    ot[:, :rows_c])
    nc.compile()
    return nc


def kernel(enc_state, dec_state, W1, b1, W2, b2, _trace=False):
    enc_state = np.ascontiguousarray(enc_state, dtype=np.float32)
    dec_state = np.ascontiguousarray(dec_state, dtype=np.float32)
    W1 = np.asarray(W1, dtype=np.float32)
    b1 = np.asarray(b1, dtype=np.float32)
    W2 = np.asarray(W2, dtype=np.float32)
    b2 = np.asarray(b2, dtype=np.float32)

    if "nc" not in _CACHE:
        _CACHE["nc"] = _build()
    nc = _CACHE["nc"]

    def chunk128(a, dt=BF16NP):
        # [n*128, w] -> [128, n*w]: partition p holds row k*128+p of chunk k
        n = a.shape[0] // 128
        return np.ascontiguousarray(
            a.reshape(n, 128, a.shape[1]).transpose(1, 0, 2).reshape(128, -1)
            .astype(dt))

    decT = chunk128(dec_state.reshape(B * U, D).T)                      # [128, 4*400]
    w1eT = chunk128(W1[:, :D].T)                                        # [128, 4*H]
    w1dT = chunk128(W1[:, D:].T)                                        # [128, 4*H]
    w2T = chunk128(W2.T)                                                # [128, 8*O]
    b1r = np.ascontiguousarray(b1.reshape(HK, 128).T)                   # [128, HK]
    b2cm = np.ascontiguousarray(b2.reshape(O // 128, 128).T)            # [128, 4]

    in_maps = []
    for c in range(NCORES):
        enc_c = enc_state[:, c * TLOC:(c + 1) * TLOC, :].reshape(PAIRS, D)
        encT_c = chunk128(enc_c.T)                                      # [128, 4*100]
        in_maps.append({
            "encT": encT_c, "decT": decT, "w1eT": w1eT, "w1dT": w1dT,
            "w2T": w2T, "b1r": b1r, "b2c": b2cm,
        })

    res = run_bass_kernel_spmd(nc, in_maps, list(range(NCORES)), trace=_trace)
    out = np.empty((B, T, U, O), dtype=np.float32)
    for c in range(NCORES):
        # device output is transposed: [O, ROWS]
        out[:, c * TLOC:(c + 1) * TLOC] = (
            res.results[c]["out"].T.reshape(B, TLOC, U, O))
    if _trace:
        kernel.last_results = res
    return out


# revision 9
# speedup vs baseline: 1.3642x; 1.0154x over previous
"""RNN-T joint network kernel for 8 Trainium2 NeuronCores.

out[b,t,u,:] = W2 @ tanh(W1e @ enc[b,t] + W1d @ dec[b,u] + b1) + b2

Shapes: B=4, T=200, U=100, D=512, H=1024, O=512 (fp32 in/out).
Sharding: T split 8 ways (25 t's per core); dec + weights replicated.

All device compute is bf16 (inputs cast on host; ~5e-4 rel err, well
under the 2e-2 gate). bf16 matmuls stream at the same 1 cycle/row as
fp32r but halve LDWEIGHTS via fast-weight-load, halve the input DMA
bytes, and remove the on-device fp32->fp32r cast pass entirely.

Per-core device program:
  Phase 1: W1 split into 4 per-dk weight tiles per side so the first
           matmul can issue as soon as the first 256KB of weights lands.
           enc matmuls run dk-outer into 8 packed psum banks (enc in
           cols 0:100, dec in cols 100:500 of the same bank); dec runs
           hk-outer so the psum->sbuf copies (+b1 on the enc half)
           pipeline behind the remaining dec matmul groups.
  Phase 2: per chunk (b, up to 5 t's): two fused broadcast-add builds
           (4 h-chunks each, [p, k, t, u] APs) on DVE -> bf16 s tile,
           one tanh over [128, 8*rows] on ACT, then 4x8 accumulating
           bf16 matmuls -> psum out^T chunks, bias-add copies split
           ACT/DVE, output DMAs split across both HWDGE rings.
"""

from contextlib import ExitStack

import numpy as np
import ml_dtypes

import concourse.bacc as bacc
import concourse.bass as bass
import concourse.mybir as mybir
import concourse.tile as tile
from concourse.bass_utils import run_bass_kernel_spmd

F32 = mybir.dt.float32
BF16 = mybir.dt.bfloat16
BF16NP = ml_dtypes.bfloat16

B, T, U, D, H, O = 4, 200, 100, 512, 1024, 512
NCORES = 8
TLOC = T // NCORES            # 25 t's per core
PAIRS = B * TLOC              # 100 (b,t) pairs per core
TCH = 5                       # t's per inner chunk
CHROWS = TCH * U              # 500 rows per chunk
NCH = TLOC // TCH             # 5 chunks per b
ROWS = PAIRS * U              # 10000 output rows per core
DK = D // 128                 # 4 contraction chunks for phase 1
HK = H // 128                 # 8 h chunks
BU = B * U                    # 400

_CACHE = {}


def _build():
    nc = bacc.Bacc("TRN2", target_bir_lowering=False, debug=False,
                   num_devices=NCORES)
    # inputs arrive pre-interleaved in SBUF layout: [128, nchunk*width],
    # partition p holding chunk k's row (k*128+p) at cols [k*width, ...)
    encT = nc.dram_tensor("encT", [128, DK * PAIRS], BF16, kind="ExternalInput")
    decT = nc.dram_tensor("decT", [128, DK * BU], BF16, kind="ExternalInput")
    w1eT = nc.dram_tensor("w1eT", [128, DK * H], BF16, kind="ExternalInput")
    w1dT = nc.dram_tensor("w1dT", [128, DK * H], BF16, kind="ExternalInput")
    w2T = nc.dram_tensor("w2T", [128, HK * O], BF16, kind="ExternalInput")
    b1r = nc.dram_tensor("b1r", [128, HK], F32, kind="ExternalInput")
    b2c = nc.dram_tensor("b2c", [128, O // 128], F32, kind="ExternalInput")
    out = nc.dram_tensor("out", [O, ROWS], F32, kind="ExternalOutput")

    with tile.TileContext(nc) as tc, ExitStack() as ctx:
        consts = ctx.enter_context(tc.tile_pool(name="consts", bufs=1))
        spool = ctx.enter_context(tc.tile_pool(name="spool", bufs=6))
        opool = ctx.enter_context(tc.tile_pool(name="opool", bufs=8))
        psB = ctx.enter_context(tc.tile_pool(name="psB", bufs=8, space="PSUM"))

        encT_s = consts.tile([128, DK * PAIRS], BF16)
        decT_s = consts.tile([128, DK * BU], BF16)
        w1e_t = [consts.tile([128, H], BF16, name=f"w1e{k}") for k in range(DK)]
        # w1d is hk-major (host re-layout): tile hk holds cols for all 4 dk's
        w1d_t = [consts.tile([128, DK * 128], BF16, name=f"w1d{k}")
                 for k in range(HK)]
        w2_s = consts.tile([128, HK * O], BF16)
        b1_s = consts.tile([128, HK], F32)
        b2c_s = consts.tile([128, O // 128], F32)
        ench_f = consts.tile([128, HK * PAIRS], BF16)
        dech_A = consts.tile([128, 4 * BU], BF16)
        dech_B = consts.tile([128, 4 * BU], BF16)

        # single-queue DMA sustains only ~160GB/s, so split the input set
        # across three queues: sync = enc path, scalar = dec path,
        # gpsimd/SWDGE = W2 (not needed until the first phase-2 group).
        nc.sync.dma_start(encT_s[:], encT[:])
        for k in range(DK):
            nc.sync.dma_start(w1e_t[k][:], w1eT[:, k * H:(k + 1) * H])
        nc.scalar.dma_start(decT_s[:], decT[:])
        for k in range(HK):
            nc.scalar.dma_start(
                w1d_t[k][:], w1dT[:, k * DK * 128:(k + 1) * DK * 128])
            if k == 1:
                nc.scalar.dma_start(b1_s[:], b1r[:])
                nc.scalar.dma_start(b2c_s[:], b2c[:])
        nc.gpsimd.dma_start(w2_s[:], w2T[:])

        # ---- phase 1 ----
        # 8 psum banks, each packing enc (cols 0:100) + dec (cols 100:500)
        # for one h-chunk. enc runs dk-outer: the dk0 matmuls only need the
        # first w1e tile, so the PE starts ~1us in.
        ph = [psB.tile([128, 512], F32, tag="psB", name=f"ph{hk}")
              for hk in range(HK)]
        for dk in range(DK):
            for hk in range(HK):
                nc.tensor.matmul(
                    ph[hk][:, 0:PAIRS],
                    lhsT=w1e_t[dk][:, hk * 128:(hk + 1) * 128],
                    rhs=encT_s[:, dk * PAIRS:(dk + 1) * PAIRS],
                    start=(dk == 0), stop=(dk == DK - 1),
                )
        for hk in range(HK):
            for dk in range(DK):
                nc.tensor.matmul(
                    ph[hk][:, PAIRS:PAIRS + BU],
                    lhsT=w1d_t[hk][:, dk * 128:(dk + 1) * 128],
                    rhs=decT_s[:, dk * BU:(dk + 1) * BU],
                    start=(dk == 0), stop=(dk == DK - 1),
                )
            # copies chase the dec matmul groups hk by hk
            nc.vector.tensor_scalar_add(
                ench_f[:, hk * PAIRS:(hk + 1) * PAIRS],
                ph[hk][:, 0:PAIRS], b1_s[:, hk:hk + 1])
            dst = dech_A if hk < 4 else dech_B
            nc.vector.tensor_copy(
                dst[:, (hk % 4) * BU:(hk % 4 + 1) * BU],
                ph[hk][:, PAIRS:PAIRS + BU])

        # ---- phase 2 ----
        ench_v = ench_f[:].rearrange("p (k t a) -> p k t a", k=HK, a=1)
        dech_vA = dech_A[:].rearrange("p (k a u) -> p k a u", k=4, a=1)
        dech_vB = dech_B[:].rearrange("p (k a u) -> p k a u", k=4, a=1)
        chunks = []
        for b in range(B):
            if b == 0:
                sizes = [1, 4, 5, 5, 5, 5]
            elif b == B - 1:
                sizes = [5, 5, 5, 5, 4, 1]
            else:
                sizes = [TCH] * NCH
            t0c = 0
            for tch in sizes:
                chunks.append((b, t0c, tch))
                t0c += tch
        def emit_half(s_t, b, t0c, tch, half):
            # broadcast-add build of 4 h-chunks, then tanh on that half.
            # A-halves (half=0) are emitted a few chunks ahead: dech_A is
            # ready mid-phase-1, so the DVE/ACT fill the pipeline while the
            # PE finishes the dec matmuls instead of starving after them.
            rows_c = tch * U
            dech_v = dech_vA if half == 0 else dech_vB
            sv = s_t[:].rearrange("p (k t u) -> p k t u", k=HK, t=TCH)
            in0 = dech_v[:, :, :, b * U:(b + 1) * U]            # [p,4,1,100]
            c0 = b * TLOC + t0c
            in1 = ench_v[:, half * 4:(half + 1) * 4, c0:c0 + tch, :]
            bc0, bc1 = bass.broadcast_tensor_aps(in0, in1)
            outap = sv[:, half * 4:(half + 1) * 4, 0:tch, :]
            nc.vector.tensor_tensor(outap, bc0, bc1, mybir.AluOpType.add)
            s_half = s_t[:].rearrange("p (k c) -> p k c", k=HK)[
                :, half * 4:(half + 1) * 4, :rows_c]
            nc.scalar.activation(s_half, s_half,
                                 mybir.ActivationFunctionType.Tanh)

        PRE = 3
        s_tiles = {}
        for ci in range(min(PRE, len(chunks))):
            b, t0c, tch = chunks[ci]
            s_tiles[ci] = spool.tile([128, HK * CHROWS], BF16, tag="s",
                                     name=f"s{ci}")
            emit_half(s_tiles[ci], b, t0c, tch, 0)
        for ci, (b, t0c, tch) in enumerate(chunks):
            rows_c = tch * U
            if ci + PRE < len(chunks):
                bn, t0n, tcn = chunks[ci + PRE]
                s_tiles[ci + PRE] = spool.tile([128, HK * CHROWS], BF16,
                                               tag="s", name=f"s{ci + PRE}")
                emit_half(s_tiles[ci + PRE], bn, t0n, tcn, 0)
            s_t = s_tiles.pop(ci)
            emit_half(s_t, b, t0c, tch, 1)
            row0 = b * (TLOC * U) + t0c * U
            # swapped matmul: W2 blocks stationary, s moving -> psum holds
            # out^T [o-chunk, rows]; b2 folds into the psum->sbuf copy.
            for oc in range(O // 128):
                ps = psB.tile([128, 512], F32, tag="psB")
                for k in range(HK):
                    nc.tensor.matmul(
                        ps[:, :rows_c],
                        lhsT=w2_s[:, k * O + oc * 128: k * O + (oc + 1) * 128],
                        rhs=s_t[:, k * CHROWS: k * CHROWS + rows_c],
                        start=(k == 0), stop=(k == HK - 1),
                    )
                ot = opool.tile([128, CHROWS], F32, tag="ot")
                if oc < 2:
                    nc.scalar.activation(
                        ot[:, :rows_c], ps[:, :rows_c],
                        mybir.ActivationFunctionType.Identity,
                        bias=b2c_s[:, oc:oc + 1])
                else:
                    nc.vector.tensor_scalar_add(
                        ot[:, :rows_c], ps[:, :rows_c], b2c_s[:, oc:oc + 1])
                ring = nc.sync if oc % 2 == 0 else nc.scalar
                ring.dma_start(
                    out[oc * 128:(oc + 1) * 128, row0:row0 + rows_c],
                    ot[:, :rows_c])
    nc.compile()
    return nc


def kernel(enc_state, dec_state, W1, b1, W2, b2, _trace=False):
    enc_state = np.ascontiguousarray(enc_state, dtype=np.float32)
    dec_state = np.ascontiguousarray(dec_state, dtype=np.float32)
    W1 = np.asarray(W1, dtype=np.float32)
    b1 = np.asarray(b1, dtype=np.float32)
    W2 = np.asarray(W2, dtype=np.float32)
    b2 = np.asarray(b2, dtype=np.float32)

    if "nc" not in _CACHE:
        _CACHE["nc"] = _build()
    nc = _CACHE["nc"]

    def chunk128(a, dt=BF16NP):
        # [n*128, w] -> [128, n*w]: partition p holds row k*128+p of chunk k
        n = a.shape[0] // 128
        return np.ascontiguousarray(
            a.reshape(n, 128, a.shape[1]).transpose(1, 0, 2).reshape(128, -1)
            .astype(dt))

    decT = chunk128(dec_state.reshape(B * U, D).T)                      # [128, 4*400]
    w1eT = chunk128(W1[:, :D].T)                                        # [128, 4*H]
    # w1d hk-major: col (hk*DK + dk)*128 + c holds dk-chunk col hk*128 + c
    w1dT = np.ascontiguousarray(
        chunk128(W1[:, D:].T).reshape(128, DK, HK, 128)
        .transpose(0, 2, 1, 3).reshape(128, DK * H))                    # [128, 8*512]
    w2T = chunk128(W2.T)                                                # [128, 8*O]
    b1r = np.ascontiguousarray(b1.reshape(HK, 128).T)                   # [128, HK]
    b2cm = np.ascontiguousarray(b2.reshape(O // 128, 128).T)            # [128, 4]

    in_maps = []
    for c in range(NCORES):
        enc_c = enc_state[:, c * TLOC:(c + 1) * TLOC, :].reshape(PAIRS, D)
        encT_c = chunk128(enc_c.T)                                      # [128, 4*100]
        in_maps.append({
            "encT": encT_c, "decT": decT, "w1eT": w1eT, "w1dT": w1dT,
            "w2T": w2T, "b1r": b1r, "b2c": b2cm,
        })

    res = run_bass_kernel_spmd(nc, in_maps, list(range(NCORES)), trace=_trace)
    out = np.empty((B, T, U, O), dtype=np.float32)
    for c in range(NCORES):
        # device output is transposed: [O, ROWS]
        out[:, c * TLOC:(c + 1) * TLOC] = (
            res.results[c]["out"].T.reshape(B, TLOC, U, O))
    if _trace:
        kernel.last_results = res
    return out
